# revision 10
# baseline (speedup 1.0000x reference)
"""ChebConv (K=2) + temporal Conv1d GNN kernel for 8 Trainium2 NeuronCores.

Strategy (data-parallel over destination nodes, channel-major on chip):
  - Node axis padded to 50176 = 392 blocks of 128; core c owns blocks
    [49c, 49c+49).
  - Host precomputes w_hat (edge weights of -D^-1/2 A D^-1/2), quantizes x
    to fp8-e4m3 rows padded to 512B (descriptor-efficient gathers), and
    sorts the edge list by (dst block, src half, dst subblock-of-32) with
    16-aligned group sizes shared across cores (max over cores).
  - Per block the device gathers fp8 source rows with SWDGE dma_gather,
    builds 32-wide one-hot*w_hat fp8 masks on DVE, and aggregates messages
    with TensorE matmuls that keep the result CHANNEL-major (x rows are the
    stationary operand), so no on-chip transposes are needed anywhere.
  - Chebyshev combine + temporal conv collapse into 14 dense 128x128
    matmuls per block with host-prefolded fp16 weights (x^T streamed from
    HBM); LeakyReLU finishes on-chip; fp16 channel-major output is
    de-transposed on the host.
"""

import numpy as np
import ml_dtypes

N = 50000
E = 1600000
W = 12
C = 32
WC = W * C            # 384
NCORES = 8
P = 128
NPAD = 50176          # 392 * 128
NB = NPAD // P        # 392
SLOTS = NB // NCORES  # 49
HALF = NPAD // 2      # 25088
GELEM = 512           # fp8 row bytes (384 data + 128 pad)
XS = 8.0              # x fp8 scale
WS = 64.0             # w_hat fp8 scale
DS = 1.0 / (XS * WS)
F8NP = ml_dtypes.float8_e4m3

_cache = {}


def _host_prep(x, A, Ew):
    src = np.asarray(A[0], np.int64)
    dst = np.asarray(A[1], np.int64)
    Ew = np.asarray(Ew, np.float32)

    deg = np.bincount(dst, weights=Ew.astype(np.float64), minlength=N).astype(np.float32)
    dinv = np.where(deg > 0, 1.0 / np.sqrt(np.maximum(deg, 1e-12)), 0.0).astype(np.float32)
    w_hat = (-dinv[src] * Ew * dinv[dst]).astype(np.float32)

    xn = np.asarray(x, np.float32).transpose(1, 0, 2).reshape(N, WC)
    xrow8 = np.zeros((NPAD, GELEM), F8NP)
    xrow8[:N, :WC] = np.clip(xn * XS, -224.0, 224.0).astype(F8NP)
    xpad = np.zeros((NPAD, WC), np.float16)
    xpad[:N] = xn
    # channel-major x for the cheb-fold matmuls: [128, 3, NPAD]
    xT = np.ascontiguousarray(xpad.T.reshape(3, 128, NPAD).transpose(1, 0, 2))

    blk = dst >> 7
    hh = (src >= HALF).astype(np.int64)
    sb = (dst >> 5) & 3
    gid = (blk * 2 + hh) * 4 + sb
    order = np.argsort(gid, kind="stable")
    loc = (src[order] - hh[order] * HALF).astype(np.int16)
    dl = (dst[order] & 127).astype(np.float16)
    wv = np.clip(w_hat[order] * WS, -224.0, 224.0).astype(np.float16)

    counts = np.bincount(gid, minlength=NB * 8)
    gstart = np.zeros(NB * 8 + 1, np.int64)
    np.cumsum(counts, out=gstart[1:])
    ccore = counts.reshape(NCORES, SLOTS, 2, 4)
    NU = ccore.max(axis=0).astype(np.int64)                       # [49, 2, 4]
    OFF = np.zeros((SLOTS, 2, 5), np.int64)
    np.cumsum(NU, axis=2, out=OFF[:, :, 1:])
    NUH = (OFF[:, :, 4] + 15) // 16 * 16                          # [49, 2]
    JH = -(-NUH // 128)                                           # [49, 2]

    ioff_flat = np.zeros(SLOTS * 2 + 1, np.int64)
    np.cumsum((NUH // 16).reshape(-1), out=ioff_flat[1:])
    IOFF = ioff_flat[:-1].reshape(SLOTS, 2)
    ITOT = int(ioff_flat[-1])
    coff_flat = np.zeros(SLOTS * 2 + 1, np.int64)
    np.cumsum(JH.reshape(-1), out=coff_flat[1:])
    CF = coff_flat[:-1].reshape(SLOTS, 2)
    JCOLTOT = int(coff_flat[-1])

    idx16 = np.zeros((NCORES, 128, ITOT), np.int16)
    dstl = np.full((NCORES, 128, JCOLTOT), 255.0, np.float16)
    what = np.zeros((NCORES, 128, JCOLTOT), np.float16)

    for c in range(NCORES):
        for i in range(SLOTS):
            for h in range(2):
                nuh = int(NUH[i, h])
                jh = int(JH[i, h])
                V = np.zeros(nuh, np.int16)
                D = np.full(jh * 128, 255.0, np.float16)
                Wv = np.zeros(jh * 128, np.float16)
                for s in range(4):
                    g = (((c * SLOTS + i) * 2 + h) * 4 + s)
                    n = int(counts[g])
                    sl = slice(int(gstart[g]), int(gstart[g]) + n)
                    o = int(OFF[i, h, s])
                    V[o:o + n] = loc[sl]
                    D[o:o + n] = dl[sl]
                    Wv[o:o + n] = wv[sl]
                io = int(IOFF[i, h])
                idx16[c, :, io:io + nuh // 16] = np.tile(V.reshape(-1, 16).T, (8, 1))
                co = int(CF[i, h])
                dstl[c, :, co:co + jh] = D.reshape(jh, 128).T
                what[c, :, co:co + jh] = Wv.reshape(jh, 128).T

    return (xrow8, xT, idx16, dstl, what, w_hat,
            NU, OFF, NUH, JH, IOFF, CF, ITOT, JCOLTOT)


def _fold_weights(Wcheb, bcheb, Wconv, bconv):
    Wcheb = np.asarray(Wcheb, np.float32)
    bcheb = np.asarray(bcheb, np.float32)
    Wconv = np.asarray(Wconv, np.float32)
    bconv = np.asarray(bconv, np.float32)
    pairs = []
    for go in range(3):
        for gi in range(max(0, go - 1), min(3, go + 2)):
            for path in range(2):
                pairs.append((path, gi, go))
    mats = np.zeros((len(pairs), 128, 128), np.float32)
    for pi, (path, gi, go) in enumerate(pairs):
        for wo in range(4 * go, 4 * go + 4):
            for k in range(3):
                wi = wo + k - 1
                if not (4 * gi <= wi < 4 * gi + 4) or not (0 <= wi < W):
                    continue
                Cmat = Wcheb[wi, path] @ Wconv[:, :, k].T  # [ci, co]
                r0 = 32 * (wi - 4 * gi)
                c0 = 32 * (wo - 4 * go)
                mats[pi, r0:r0 + 32, c0:c0 + 32] = Cmat
    mats_sb = np.ascontiguousarray(
        mats.transpose(1, 0, 2).reshape(128, -1)).astype(np.float16)
    bias = np.zeros((12, 32), np.float32)
    for wo in range(12):
        bias[wo] = bconv.copy()
        for k in range(3):
            wi = wo + k - 1
            if 0 <= wi < W:
                bias[wo] += bcheb[wi] @ Wconv[:, :, k].T
    bias_sb = bias.reshape(3, 128).T.copy()  # [128, 3]
    return mats_sb, bias_sb, pairs


def _build_program(NU, OFF, NUH, JH, IOFF, CF, ITOT, JCOLTOT, n_pairs):
    import concourse.bacc as bacc
    import concourse.tile as tile
    from concourse import mybir

    nc = bacc.Bacc("TRN2", target_bir_lowering=False, debug=False,
                   num_devices=NCORES)
    f16, f32, i16 = mybir.dt.float16, mybir.dt.float32, mybir.dt.int16
    f8 = mybir.dt.float8e4
    xrow8 = nc.dram_tensor("xrow8", [NPAD, GELEM], f8, kind="ExternalInput")
    xTd = nc.dram_tensor("xT", [128, 3, SLOTS * P], f16, kind="ExternalInput")
    idxd = nc.dram_tensor("idx16", [128, ITOT], i16, kind="ExternalInput")
    dstld = nc.dram_tensor("dstl", [128, JCOLTOT], f16, kind="ExternalInput")
    whatd = nc.dram_tensor("what", [128, JCOLTOT], f16, kind="ExternalInput")
    matsd = nc.dram_tensor("mats", [128, n_pairs * 128], f16, kind="ExternalInput")
    biasd = nc.dram_tensor("biasd", [128, 3], f32, kind="ExternalInput")
    iotad = nc.dram_tensor("iota", [128, 128], f16, kind="ExternalInput")
    out_pc = nc.dram_tensor("out_pc", [128, 3, SLOTS * P], f16, kind="ExternalOutput")

    pairs_by_go = [[], [], []]
    pi = 0
    for go in range(3):
        for gi in range(max(0, go - 1), min(3, go + 2)):
            for path in range(2):
                pairs_by_go[go].append((pi, gi, path))
                pi += 1

    JSMAX = int((JH[:, 0] + JH[:, 1]).max())
    # per-slot (h, s) one-hot column ranges
    WMX = 0
    WSMAX = 0
    for i in range(SLOTS):
        wtot = 0
        for h in range(2):
            for s in range(4):
                if NU[i, h, s] == 0:
                    continue
                o0, o1 = int(OFF[i, h, s]), int(OFF[i, h, s] + NU[i, h, s] - 1)
                wn = o1 // 128 - o0 // 128 + 1
                wtot += wn
                WSMAX = max(WSMAX, wn)
        WMX = max(WMX, wtot)

    with tile.TileContext(nc) as tc:
        with tc.tile_pool(name="const", bufs=1) as cp, \
             tc.tile_pool(name="xgp", bufs=2) as xgp, \
             tc.tile_pool(name="wmp", bufs=2) as wmp, \
             tc.tile_pool(name="eqp", bufs=2) as eqp, \
             tc.tile_pool(name="t1p", bufs=2) as t1p, \
             tc.tile_pool(name="tlp", bufs=2) as tlp, \
             tc.tile_pool(name="stp", bufs=2) as stp, \
             tc.tile_pool(name="pst1", bufs=2, space="PSUM") as pst1, \
             tc.tile_pool(name="psy", bufs=2, space="PSUM") as psy:
            # small idx slice for slots 0-1 first so their gathers start
            # immediately; the big loads all queue behind them.
            S01 = int(IOFF[2, 0])
            idx0_t = cp.tile([128, S01], i16)
            nc.sync.dma_start(out=idx0_t[:], in_=idxd.ap()[:, :S01])
            idx_t = cp.tile([128, ITOT], i16)
            nc.sync.dma_start(out=idx_t[:], in_=idxd.ap())
            zero3 = cp.tile([128, 3, 128], f8)
            nc.vector.memset(zero3[:], 0.0)
            mats_t = cp.tile([128, n_pairs * 128], f16)
            bias_t = cp.tile([128, 3], f32)
            iota_t = cp.tile([128, 128], f16)
            dm_t = cp.tile([128, JCOLTOT], f16)
            wh_t = cp.tile([128, JCOLTOT], f16)
            xt_t = cp.tile([128, 3, SLOTS * P], f16)

            stage_t = None
            for i in range(SLOTS):
                xg_t = xgp.tile([128, JSMAX, GELEM], f8, tag="xg")
                for h in range(2):
                    ch = 0 if h == 0 else int(JH[i, 0])
                    nexact = int(OFF[i, h, 4])
                    io = int(IOFF[i, h])
                    it = idx0_t if i < 2 else idx_t
                    nc.gpsimd.dma_gather(
                        xg_t[:, ch:ch + int(JH[i, h]), :],
                        xrow8.ap()[h * HALF:(h + 1) * HALF, :],
                        it[:, io:io + (nexact + 15) // 16],
                        nexact, nexact, GELEM,
                        single_packet=False)
                if i == 0:
                    nc.sync.dma_start(out=dm_t[:], in_=dstld.ap())
                    nc.sync.dma_start(out=wh_t[:], in_=whatd.ap())
                    nc.sync.dma_start(out=iota_t[:], in_=iotad.ap())
                    nc.sync.dma_start(out=mats_t[:], in_=matsd.ap())
                    nc.sync.dma_start(out=bias_t[:], in_=biasd.ap())
                    nc.sync.dma_start(out=xt_t[:], in_=xTd.ap())
                if i % 7 == 0:
                    stage_t = stp.tile([128, 3, 7 * P], f16, tag="st")

                # one-hot * w_hat masks (fp8), one region per (h, s)
                wm_t = wmp.tile([128, WMX, 32], f8, tag="wm")
                woff = 0
                mmlist = []  # (xg col, wm col, s, contraction rows)
                for h in range(2):
                    chb = 0 if h == 0 else int(JH[i, 0])
                    nexact = int(OFF[i, h, 4])
                    lastc = chb + (nexact - 1) // 128
                    rem = nexact % 128
                    for s in range(4):
                        if NU[i, h, s] == 0:
                            continue
                        o0 = int(OFF[i, h, s])
                        o1 = o0 + int(NU[i, h, s]) - 1
                        c0, c1 = o0 // 128, o1 // 128
                        wn = c1 - c0 + 1
                        a = int(CF[i, h]) + c0
                        eq_t = eqp.tile([128, WSMAX, 32], f16, tag="eq")
                        nc.vector.tensor_tensor(
                            out=eq_t[:, :wn, :],
                            in0=dm_t[:, a:a + wn].unsqueeze(2).to_broadcast([128, wn, 32]),
                            in1=iota_t[:, 32 * s:32 * s + 32].unsqueeze(1).to_broadcast([128, wn, 32]),
                            op=mybir.AluOpType.is_equal)
                        nc.vector.tensor_tensor(
                            out=wm_t[:, woff:woff + wn, :],
                            in0=eq_t[:, :wn, :],
                            in1=wh_t[:, a:a + wn].unsqueeze(2).to_broadcast([128, wn, 32]),
                            op=mybir.AluOpType.mult)
                        for q in range(wn):
                            xc = chb + c0 + q
                            pr = rem if (xc == lastc and rem) else 128
                            mmlist.append((xc, woff + q, s, pr))
                        woff += wn

                # message aggregation, channel-major: t1T[ch, dst] in PSUM
                pst = pst1.tile([128, 3, 128], f32, space="PSUM", tag="t1")
                nc.tensor.matmul(out=pst[:], lhsT=zero3[:, 0, :], rhs=zero3[:],
                                 start=True, stop=False, skip_group_check=True)
                total = 3 * len(mmlist)
                k = 0
                for b in range(3):
                    for (xcol, wcol, s, pr) in mmlist:
                        k += 1
                        nc.tensor.matmul(
                            out=pst[:, b, 32 * s:32 * s + 32],
                            lhsT=xg_t[0:pr, xcol, b * 128:(b + 1) * 128],
                            rhs=wm_t[0:pr, wcol, :],
                            start=False, stop=(k == total),
                            skip_group_check=True)

                t1sb = t1p.tile([128, 3, 128], f16, tag="t1sb")
                nc.scalar.mul(out=t1sb[:], in_=pst[:], mul=DS)

                # cheb + temporal-conv fold (channel-major y)
                yps = psy.tile([128, 3, 128], f32, space="PSUM", tag="y")
                for go in range(3):
                    plist = pairs_by_go[go]
                    for n_, (pi_, gi, path) in enumerate(plist):
                        rhs = (xt_t[:, gi, i * P:(i + 1) * P] if path == 0
                               else t1sb[:, gi, :])
                        nc.tensor.matmul(
                            out=yps[:, go, :],
                            lhsT=mats_t[:, pi_ * 128:(pi_ + 1) * 128],
                            rhs=rhs,
                            start=(n_ == 0), stop=(n_ == len(plist) - 1),
                            skip_group_check=True)
                    ysl = stage_t[:, go, (i % 7) * P:(i % 7 + 1) * P]
                    nc.scalar.activation(out=ysl, in_=yps[:, go, :],
                                         func=mybir.ActivationFunctionType.Identity,
                                         bias=bias_t[:, go:go + 1], scale=1.0)
                    tl = tlp.tile([128, 128], f16, tag="tl")
                    nc.vector.tensor_scalar_mul(out=tl[:], in0=ysl, scalar1=0.01)
                    nc.vector.tensor_tensor(out=ysl, in0=ysl, in1=tl[:],
                                            op=mybir.AluOpType.max)

                if i % 7 == 6:
                    nc.sync.dma_start(
                        out=out_pc.ap()[:, :, (i - 6) * P:(i + 1) * P],
                        in_=stage_t[:])

    nc.compile()
    return nc


def kernel(x, A, Ew, Wcheb, bcheb, Wconv, bconv, batch_size=1):
    from concourse.bass_utils import run_bass_kernel_spmd

    (xrow8, xT, idx16, dstl, what, w_hat,
     NU, OFF, NUH, JH, IOFF, CF, ITOT, JCOLTOT) = _host_prep(x, A, Ew)
    mats_sb, bias_sb, pairs = _fold_weights(Wcheb, bcheb, Wconv, bconv)

    key = (ITOT, JCOLTOT, tuple(NU.reshape(-1).tolist()))
    if key not in _cache:
        _cache[key] = _build_program(NU, OFF, NUH, JH, IOFF, CF,
                                     ITOT, JCOLTOT, len(pairs))
    nc = _cache[key]

    iota_np = np.tile(np.arange(128, dtype=np.float16)[None, :], (128, 1))
    in_maps = []
    for c in range(NCORES):
        in_maps.append(dict(
            xrow8=xrow8,
            xT=np.ascontiguousarray(xT[:, :, c * SLOTS * P:(c + 1) * SLOTS * P]),
            idx16=idx16[c], dstl=dstl[c], what=what[c],
            mats=mats_sb, biasd=bias_sb, iota=iota_np))
    res = run_bass_kernel_spmd(nc, in_maps, core_ids=list(range(NCORES)))
    # out_pc[c]: [128, 3, 6272] channel-major fp16 -> [50000, 12, 32] f32
    cols = [np.asarray(res.results[c]["out_pc"], np.float16).reshape(128, 3, SLOTS * P)
            for c in range(NCORES)]
    full = np.concatenate(cols, axis=2)                  # [128, 3, 50176]
    full = full.transpose(1, 0, 2).reshape(WC, NPAD)     # [384, 50176]
    y = np.ascontiguousarray(full[:, :N].T).astype(np.float32)  # [N, 384]
    return y.reshape(N, W, C)


# revision 14
# speedup vs baseline: 1.0008x; 1.0008x over previous
"""ChebConv (K=2) + temporal Conv1d GNN kernel for 8 Trainium2 NeuronCores.

Strategy (data-parallel over destination nodes, channel-major on chip):
  - Node axis padded to 50176 = 392 blocks of 128; core c owns blocks
    [49c, 49c+49).
  - Host precomputes w_hat (edge weights of -D^-1/2 A D^-1/2), quantizes x
    to fp8-e4m3 rows padded to 512B (descriptor-efficient gathers), and
    sorts the edge list by (dst block, src half, dst subblock-of-32) with
    16-aligned group sizes shared across cores (max over cores).
  - Per block the device gathers fp8 source rows with SWDGE dma_gather,
    builds 32-wide one-hot*w_hat fp8 masks on DVE, and aggregates messages
    with TensorE matmuls that keep the result CHANNEL-major (x rows are the
    stationary operand), so no on-chip transposes are needed anywhere.
  - Chebyshev combine + temporal conv collapse into 14 dense 128x128
    matmuls per block with host-prefolded fp16 weights (x^T streamed from
    HBM); LeakyReLU finishes on-chip; fp16 channel-major output is
    de-transposed on the host.
"""

import numpy as np
import ml_dtypes

N = 50000
E = 1600000
W = 12
C = 32
WC = W * C            # 384
NCORES = 8
P = 128
NPAD = 50176          # 392 * 128
NB = NPAD // P        # 392
SLOTS = NB // NCORES  # 49
HALF = NPAD // 2      # 25088
GELEM = 512           # fp8 row bytes (384 data + 128 pad)
XS = 8.0              # x fp8 scale
WS = 64.0             # w_hat fp8 scale
DS = 1.0 / (XS * WS)
F8NP = ml_dtypes.float8_e4m3

_cache = {}


def _host_prep(x, A, Ew):
    src = np.asarray(A[0], np.int64)
    dst = np.asarray(A[1], np.int64)
    Ew = np.asarray(Ew, np.float32)

    deg = np.bincount(dst, weights=Ew.astype(np.float64), minlength=N).astype(np.float32)
    dinv = np.where(deg > 0, 1.0 / np.sqrt(np.maximum(deg, 1e-12)), 0.0).astype(np.float32)
    w_hat = (-dinv[src] * Ew * dinv[dst]).astype(np.float32)

    xn = np.asarray(x, np.float32).transpose(1, 0, 2).reshape(N, WC)
    xrow8 = np.zeros((NPAD, GELEM), F8NP)
    xrow8[:N, :WC] = np.clip(xn * XS, -224.0, 224.0).astype(F8NP)
    xpad = np.zeros((NPAD, WC), np.float16)
    xpad[:N] = xn
    # channel-major x for the cheb-fold matmuls: [128, 3, NPAD]
    xT = np.ascontiguousarray(xpad.T.reshape(3, 128, NPAD).transpose(1, 0, 2))

    blk = dst >> 7
    hh = (src >= HALF).astype(np.int64)
    sb = (dst >> 5) & 3
    gid = (blk * 2 + hh) * 4 + sb
    order = np.argsort(gid, kind="stable")
    loc = (src[order] - hh[order] * HALF).astype(np.int16)
    dl = (dst[order] & 127).astype(np.float16)
    wv = np.clip(w_hat[order] * WS, -224.0, 224.0).astype(np.float16)

    counts = np.bincount(gid, minlength=NB * 8)
    gstart = np.zeros(NB * 8 + 1, np.int64)
    np.cumsum(counts, out=gstart[1:])
    ccore = counts.reshape(NCORES, SLOTS, 2, 4)
    NU = ccore.max(axis=0).astype(np.int64)                       # [49, 2, 4]
    OFF = np.zeros((SLOTS, 2, 5), np.int64)
    np.cumsum(NU, axis=2, out=OFF[:, :, 1:])
    NUH = (OFF[:, :, 4] + 15) // 16 * 16                          # [49, 2]
    JH = -(-NUH // 128)                                           # [49, 2]

    ioff_flat = np.zeros(SLOTS * 2 + 1, np.int64)
    np.cumsum((NUH // 16).reshape(-1), out=ioff_flat[1:])
    IOFF = ioff_flat[:-1].reshape(SLOTS, 2)
    ITOT = int(ioff_flat[-1])
    coff_flat = np.zeros(SLOTS * 2 + 1, np.int64)
    np.cumsum(JH.reshape(-1), out=coff_flat[1:])
    CF = coff_flat[:-1].reshape(SLOTS, 2)
    JCOLTOT = int(coff_flat[-1])

    idx16 = np.zeros((NCORES, 128, ITOT), np.int16)
    dstl = np.full((NCORES, 128, JCOLTOT), 255.0, np.float16)
    what = np.zeros((NCORES, 128, JCOLTOT), np.float16)

    for c in range(NCORES):
        for i in range(SLOTS):
            for h in range(2):
                nuh = int(NUH[i, h])
                jh = int(JH[i, h])
                V = np.zeros(nuh, np.int16)
                D = np.full(jh * 128, 255.0, np.float16)
                Wv = np.zeros(jh * 128, np.float16)
                for s in range(4):
                    g = (((c * SLOTS + i) * 2 + h) * 4 + s)
                    n = int(counts[g])
                    sl = slice(int(gstart[g]), int(gstart[g]) + n)
                    o = int(OFF[i, h, s])
                    V[o:o + n] = loc[sl]
                    D[o:o + n] = dl[sl]
                    Wv[o:o + n] = wv[sl]
                io = int(IOFF[i, h])
                idx16[c, :, io:io + nuh // 16] = np.tile(V.reshape(-1, 16).T, (8, 1))
                co = int(CF[i, h])
                dstl[c, :, co:co + jh] = D.reshape(jh, 128).T
                what[c, :, co:co + jh] = Wv.reshape(jh, 128).T

    return (xrow8, xT, idx16, dstl, what, w_hat,
            NU, OFF, NUH, JH, IOFF, CF, ITOT, JCOLTOT)


def _fold_weights(Wcheb, bcheb, Wconv, bconv):
    Wcheb = np.asarray(Wcheb, np.float32)
    bcheb = np.asarray(bcheb, np.float32)
    Wconv = np.asarray(Wconv, np.float32)
    bconv = np.asarray(bconv, np.float32)
    pairs = []
    for go in range(3):
        for gi in range(max(0, go - 1), min(3, go + 2)):
            for path in range(2):
                pairs.append((path, gi, go))
    mats = np.zeros((len(pairs), 128, 128), np.float32)
    for pi, (path, gi, go) in enumerate(pairs):
        for wo in range(4 * go, 4 * go + 4):
            for k in range(3):
                wi = wo + k - 1
                if not (4 * gi <= wi < 4 * gi + 4) or not (0 <= wi < W):
                    continue
                Cmat = Wcheb[wi, path] @ Wconv[:, :, k].T  # [ci, co]
                r0 = 32 * (wi - 4 * gi)
                c0 = 32 * (wo - 4 * go)
                mats[pi, r0:r0 + 32, c0:c0 + 32] = Cmat
    mats_sb = np.ascontiguousarray(
        mats.transpose(1, 0, 2).reshape(128, -1)).astype(np.float16)
    bias = np.zeros((12, 32), np.float32)
    for wo in range(12):
        bias[wo] = bconv.copy()
        for k in range(3):
            wi = wo + k - 1
            if 0 <= wi < W:
                bias[wo] += bcheb[wi] @ Wconv[:, :, k].T
    bias_sb = bias.reshape(3, 128).T.copy()  # [128, 3]
    return mats_sb, bias_sb, pairs


def _build_program(NU, OFF, NUH, JH, IOFF, CF, ITOT, JCOLTOT, n_pairs):
    import concourse.bacc as bacc
    import concourse.tile as tile
    from concourse import mybir

    nc = bacc.Bacc("TRN2", target_bir_lowering=False, debug=False,
                   num_devices=NCORES)
    f16, f32, i16 = mybir.dt.float16, mybir.dt.float32, mybir.dt.int16
    f8 = mybir.dt.float8e4
    xrow8 = nc.dram_tensor("xrow8", [NPAD, GELEM], f8, kind="ExternalInput")
    xTd = nc.dram_tensor("xT", [128, 3, SLOTS * P], f16, kind="ExternalInput")
    idxd = nc.dram_tensor("idx16", [128, ITOT], i16, kind="ExternalInput")
    dstld = nc.dram_tensor("dstl", [128, JCOLTOT], f16, kind="ExternalInput")
    whatd = nc.dram_tensor("what", [128, JCOLTOT], f16, kind="ExternalInput")
    matsd = nc.dram_tensor("mats", [128, n_pairs * 128], f16, kind="ExternalInput")
    biasd = nc.dram_tensor("biasd", [128, 3], f32, kind="ExternalInput")
    iotad = nc.dram_tensor("iota", [128, 128], f16, kind="ExternalInput")
    out_pc = nc.dram_tensor("out_pc", [128, 3, SLOTS * P], f16, kind="ExternalOutput")

    pairs_by_go = [[], [], []]
    pi = 0
    for go in range(3):
        for gi in range(max(0, go - 1), min(3, go + 2)):
            for path in range(2):
                pairs_by_go[go].append((pi, gi, path))
                pi += 1

    JSMAX = int((JH[:, 0] + JH[:, 1]).max())
    # per-slot (h, s) one-hot column ranges
    WMX = 0
    WSMAX = 0
    for i in range(SLOTS):
        wtot = 0
        for h in range(2):
            for s in range(4):
                if NU[i, h, s] == 0:
                    continue
                o0, o1 = int(OFF[i, h, s]), int(OFF[i, h, s] + NU[i, h, s] - 1)
                wn = o1 // 128 - o0 // 128 + 1
                wtot += wn
                WSMAX = max(WSMAX, wn)
        WMX = max(WMX, wtot)

    with tile.TileContext(nc) as tc:
        with tc.tile_pool(name="const", bufs=1) as cp, \
             tc.tile_pool(name="xgp", bufs=2) as xgp, \
             tc.tile_pool(name="wmp", bufs=2) as wmp, \
             tc.tile_pool(name="eqp", bufs=2) as eqp, \
             tc.tile_pool(name="t1p", bufs=2) as t1p, \
             tc.tile_pool(name="tlp", bufs=2) as tlp, \
             tc.tile_pool(name="stp", bufs=2) as stp, \
             tc.tile_pool(name="pst1", bufs=2, space="PSUM") as pst1, \
             tc.tile_pool(name="psy", bufs=2, space="PSUM") as psy:
            # small idx slice for slots 0-1 first so their gathers start
            # immediately; the big loads all queue behind them.
            S01 = int(IOFF[2, 0])
            idx0_t = cp.tile([128, S01], i16)
            nc.sync.dma_start(out=idx0_t[:], in_=idxd.ap()[:, :S01])
            idx_t = cp.tile([128, ITOT], i16)
            nc.sync.dma_start(out=idx_t[:], in_=idxd.ap())
            zero3 = cp.tile([128, 3, 128], f8)
            nc.vector.memset(zero3[:], 0.0)
            mats_t = cp.tile([128, n_pairs * 128], f16)
            bias_t = cp.tile([128, 3], f32)
            iota_t = cp.tile([128, 128], f16)
            dm_t = cp.tile([128, JCOLTOT], f16)
            wh_t = cp.tile([128, JCOLTOT], f16)
            xt_t = cp.tile([128, 3, SLOTS * P], f16)

            # 7-slot output batches, with small final batches so the last
            # slots' compute tail overlaps earlier writes
            bstart = {0: 7, 7: 7, 14: 7, 21: 7, 28: 7, 35: 7,
                      42: 3, 45: 2, 47: 1, 48: 1}
            stage_t = None
            bs, bw = 0, 7
            for i in range(SLOTS):
                xg_t = xgp.tile([128, JSMAX, GELEM], f8, tag="xg")
                for h in range(2):
                    ch = 0 if h == 0 else int(JH[i, 0])
                    nexact = int(OFF[i, h, 4])
                    io = int(IOFF[i, h])
                    it = idx0_t if i < 2 else idx_t
                    nc.gpsimd.dma_gather(
                        xg_t[:, ch:ch + int(JH[i, h]), :],
                        xrow8.ap()[h * HALF:(h + 1) * HALF, :],
                        it[:, io:io + (nexact + 15) // 16],
                        nexact, nexact, GELEM,
                        single_packet=False)
                if i == 0:
                    nc.sync.dma_start(out=dm_t[:], in_=dstld.ap())
                    nc.sync.dma_start(out=wh_t[:], in_=whatd.ap())
                    nc.sync.dma_start(out=iota_t[:], in_=iotad.ap())
                    nc.sync.dma_start(out=mats_t[:], in_=matsd.ap())
                    nc.sync.dma_start(out=bias_t[:], in_=biasd.ap())
                    nc.sync.dma_start(out=xt_t[:], in_=xTd.ap())
                if i in bstart:
                    bs, bw = i, bstart[i]
                    stage_t = stp.tile([128, 3, 7 * P], f16, tag="st")

                # one-hot * w_hat masks (fp16 so w_hat stays exact; the
                # message matmul mixes fp16 moving x fp8 stationary)
                wm_t = wmp.tile([128, WMX, 32], f16, tag="wm")
                woff = 0
                mmlist = []  # (xg col, wm col, s, contraction rows)
                for h in range(2):
                    chb = 0 if h == 0 else int(JH[i, 0])
                    nexact = int(OFF[i, h, 4])
                    lastc = chb + (nexact - 1) // 128
                    rem = nexact % 128
                    for s in range(4):
                        if NU[i, h, s] == 0:
                            continue
                        o0 = int(OFF[i, h, s])
                        o1 = o0 + int(NU[i, h, s]) - 1
                        c0, c1 = o0 // 128, o1 // 128
                        wn = c1 - c0 + 1
                        a = int(CF[i, h]) + c0
                        eq_t = eqp.tile([128, WSMAX, 32], f16, tag="eq")
                        nc.vector.tensor_tensor(
                            out=eq_t[:, :wn, :],
                            in0=dm_t[:, a:a + wn].unsqueeze(2).to_broadcast([128, wn, 32]),
                            in1=iota_t[:, 32 * s:32 * s + 32].unsqueeze(1).to_broadcast([128, wn, 32]),
                            op=mybir.AluOpType.is_equal)
                        nc.vector.tensor_tensor(
                            out=wm_t[:, woff:woff + wn, :],
                            in0=eq_t[:, :wn, :],
                            in1=wh_t[:, a:a + wn].unsqueeze(2).to_broadcast([128, wn, 32]),
                            op=mybir.AluOpType.mult)
                        for q in range(wn):
                            xc = chb + c0 + q
                            pr = rem if (xc == lastc and rem) else 128
                            mmlist.append((xc, woff + q, s, pr))
                        woff += wn

                # message aggregation, channel-major: t1T[ch, dst] in PSUM
                pst = pst1.tile([128, 3, 128], f32, space="PSUM", tag="t1")
                nc.tensor.matmul(out=pst[:], lhsT=zero3[:, 0, :], rhs=zero3[:],
                                 start=True, stop=False, skip_group_check=True)
                total = 3 * len(mmlist)
                k = 0
                for b in range(3):
                    for (xcol, wcol, s, pr) in mmlist:
                        k += 1
                        nc.tensor.matmul(
                            out=pst[:, b, 32 * s:32 * s + 32],
                            lhsT=xg_t[0:pr, xcol, b * 128:(b + 1) * 128],
                            rhs=wm_t[0:pr, wcol, :],
                            start=False, stop=(k == total),
                            skip_group_check=True)

                t1sb = t1p.tile([128, 3, 128], f16, tag="t1sb")
                nc.scalar.mul(out=t1sb[:], in_=pst[:], mul=DS)

                # cheb + temporal-conv fold (channel-major y)
                yps = psy.tile([128, 3, 128], f32, space="PSUM", tag="y")
                for go in range(3):
                    plist = pairs_by_go[go]
                    for n_, (pi_, gi, path) in enumerate(plist):
                        rhs = (xt_t[:, gi, i * P:(i + 1) * P] if path == 0
                               else t1sb[:, gi, :])
                        nc.tensor.matmul(
                            out=yps[:, go, :],
                            lhsT=mats_t[:, pi_ * 128:(pi_ + 1) * 128],
                            rhs=rhs,
                            start=(n_ == 0), stop=(n_ == len(plist) - 1),
                            skip_group_check=True)
                    ysl = stage_t[:, go, (i - bs) * P:(i - bs + 1) * P]
                    nc.scalar.activation(out=ysl, in_=yps[:, go, :],
                                         func=mybir.ActivationFunctionType.Identity,
                                         bias=bias_t[:, go:go + 1], scale=1.0)
                    tl = tlp.tile([128, 128], f16, tag="tl")
                    nc.vector.tensor_scalar_mul(out=tl[:], in0=ysl, scalar1=0.01)
                    nc.vector.tensor_tensor(out=ysl, in0=ysl, in1=tl[:],
                                            op=mybir.AluOpType.max)

                if i == bs + bw - 1:
                    nc.sync.dma_start(
                        out=out_pc.ap()[:, :, bs * P:(bs + bw) * P],
                        in_=stage_t[:, :, 0:bw * P])

    nc.compile()
    return nc


def kernel(x, A, Ew, Wcheb, bcheb, Wconv, bconv, batch_size=1):
    from concourse.bass_utils import run_bass_kernel_spmd

    (xrow8, xT, idx16, dstl, what, w_hat,
     NU, OFF, NUH, JH, IOFF, CF, ITOT, JCOLTOT) = _host_prep(x, A, Ew)
    mats_sb, bias_sb, pairs = _fold_weights(Wcheb, bcheb, Wconv, bconv)

    key = (ITOT, JCOLTOT, tuple(NU.reshape(-1).tolist()))
    if key not in _cache:
        _cache[key] = _build_program(NU, OFF, NUH, JH, IOFF, CF,
                                     ITOT, JCOLTOT, len(pairs))
    nc = _cache[key]

    iota_np = np.tile(np.arange(128, dtype=np.float16)[None, :], (128, 1))
    in_maps = []
    for c in range(NCORES):
        in_maps.append(dict(
            xrow8=xrow8,
            xT=np.ascontiguousarray(xT[:, :, c * SLOTS * P:(c + 1) * SLOTS * P]),
            idx16=idx16[c], dstl=dstl[c], what=what[c],
            mats=mats_sb, biasd=bias_sb, iota=iota_np))
    res = run_bass_kernel_spmd(nc, in_maps, core_ids=list(range(NCORES)))
    # out_pc[c]: [128, 3, 6272] channel-major fp16 -> [50000, 12, 32] f32
    cols = [np.asarray(res.results[c]["out_pc"], np.float16).reshape(128, 3, SLOTS * P)
            for c in range(NCORES)]
    full = np.concatenate(cols, axis=2)                  # [128, 3, 50176]
    full = full.transpose(1, 0, 2).reshape(WC, NPAD)     # [384, 50176]
    y = np.ascontiguousarray(full[:, :N].T).astype(np.float32)  # [N, 384]
    return y.reshape(N, W, C)


# revision 21
# speedup vs baseline: 1.0034x; 1.0025x over previous
"""ChebConv (K=2) + temporal Conv1d GNN kernel for 8 Trainium2 NeuronCores.

Strategy (data-parallel over destination nodes, channel-major on chip):
  - Node axis padded to 50176 = 392 blocks of 128; core c owns blocks
    [49c, 49c+49).
  - Host precomputes w_hat (edge weights of -D^-1/2 A D^-1/2), quantizes x
    to fp8-e4m3 rows padded to 512B (descriptor-efficient gathers), and
    sorts the edge list by (dst block, src half, dst subblock-of-32) with
    16-aligned group sizes shared across cores (max over cores).
  - Per block the device gathers fp8 source rows with SWDGE dma_gather,
    builds 32-wide one-hot*w_hat fp8 masks on DVE, and aggregates messages
    with TensorE matmuls that keep the result CHANNEL-major (x rows are the
    stationary operand), so no on-chip transposes are needed anywhere.
  - Chebyshev combine + temporal conv collapse into 14 dense 128x128
    matmuls per block with host-prefolded fp16 weights (x^T streamed from
    HBM); LeakyReLU finishes on-chip; fp16 channel-major output is
    de-transposed on the host.
"""

import numpy as np
import ml_dtypes

N = 50000
E = 1600000
W = 12
C = 32
WC = W * C            # 384
NCORES = 8
P = 128
NPAD = 50176          # 392 * 128
NB = NPAD // P        # 392
SLOTS = NB // NCORES  # 49
HALF = NPAD // 2      # 25088
GELEM = 512           # fp8 row bytes (384 data + 128 pad)
XS = 8.0              # x fp8 scale
WS = 64.0             # w_hat fp8 scale
DS = 1.0 / (XS * WS)
F8NP = ml_dtypes.float8_e4m3

_cache = {}


def _host_prep(x, A, Ew):
    src = np.asarray(A[0], np.int64)
    dst = np.asarray(A[1], np.int64)
    Ew = np.asarray(Ew, np.float32)

    deg = np.bincount(dst, weights=Ew.astype(np.float64), minlength=N).astype(np.float32)
    dinv = np.where(deg > 0, 1.0 / np.sqrt(np.maximum(deg, 1e-12)), 0.0).astype(np.float32)
    w_hat = (-dinv[src] * Ew * dinv[dst]).astype(np.float32)

    xn = np.asarray(x, np.float32).transpose(1, 0, 2).reshape(N, WC)
    xrow8 = np.zeros((NPAD, GELEM), F8NP)
    xrow8[:N, :WC] = np.clip(xn * XS, -224.0, 224.0).astype(F8NP)
    xpad = np.zeros((NPAD, WC), np.float16)
    xpad[:N] = xn
    # channel-major x for the cheb-fold matmuls: [128, 3, NPAD]
    xT = np.ascontiguousarray(xpad.T.reshape(3, 128, NPAD).transpose(1, 0, 2))

    blk = dst >> 7
    hh = (src >= HALF).astype(np.int64)
    sb = (dst >> 5) & 3
    gid = (blk * 2 + hh) * 4 + sb
    order = np.argsort(gid, kind="stable")
    loc = (src[order] - hh[order] * HALF).astype(np.int16)
    dl = (dst[order] & 127).astype(np.float16)
    wv = np.clip(w_hat[order] * WS, -224.0, 224.0).astype(np.float16)

    counts = np.bincount(gid, minlength=NB * 8)
    gstart = np.zeros(NB * 8 + 1, np.int64)
    np.cumsum(counts, out=gstart[1:])
    ccore = counts.reshape(NCORES, SLOTS, 2, 4)
    NU = ccore.max(axis=0).astype(np.int64)                       # [49, 2, 4]

    # per-slot gather-call list; the last slots use split calls so their
    # message matmuls can chase partial gathers (shorter pipeline tail)
    CALLS = []
    for i in range(SLOTS):
        if i >= SLOTS - 2:
            CALLS.append([(0, 0, 2), (0, 2, 4), (1, 0, 2), (1, 2, 4)])
        else:
            CALLS.append([(0, 0, 4), (1, 0, 4)])

    # per-call static layout: exact index count, 16-aligned idx span,
    # column count / offsets, and per-s offsets within the call
    meta = []   # per slot: list of dicts
    io = co = 0
    for i in range(SLOTS):
        mslot = []
        ch = 0
        for (h, slo, shi) in CALLS[i]:
            offs = np.zeros(5, np.int64)
            np.cumsum(NU[i, h, slo:shi], out=offs[1:shi - slo + 1])
            nexact = int(offs[shi - slo])
            n16 = (nexact + 15) // 16 * 16
            jc = -(-n16 // 128)
            mslot.append(dict(h=h, slo=slo, shi=shi, offs=offs,
                              nexact=nexact, n16=n16, jc=jc,
                              io=io, co=co, ch=ch))
            io += n16 // 16
            co += jc
            ch += jc
        meta.append(mslot)
    ITOT = io
    JCOLTOT = co

    idx16 = np.zeros((NCORES, 128, ITOT), np.int16)
    dstl = np.full((NCORES, 128, JCOLTOT), 255.0, np.float16)
    what = np.zeros((NCORES, 128, JCOLTOT), np.float16)

    for c in range(NCORES):
        for i in range(SLOTS):
            for m in meta[i]:
                n16, jc = m["n16"], m["jc"]
                V = np.zeros(n16, np.int16)
                D = np.full(jc * 128, 255.0, np.float16)
                Wv = np.zeros(jc * 128, np.float16)
                for s in range(m["slo"], m["shi"]):
                    g = (((c * SLOTS + i) * 2 + m["h"]) * 4 + s)
                    n = int(counts[g])
                    sl = slice(int(gstart[g]), int(gstart[g]) + n)
                    o = int(m["offs"][s - m["slo"]])
                    V[o:o + n] = loc[sl]
                    D[o:o + n] = dl[sl]
                    Wv[o:o + n] = wv[sl]
                idx16[c, :, m["io"]:m["io"] + n16 // 16] = \
                    np.tile(V.reshape(-1, 16).T, (8, 1))
                dstl[c, :, m["co"]:m["co"] + jc] = D.reshape(jc, 128).T
                what[c, :, m["co"]:m["co"] + jc] = Wv.reshape(jc, 128).T

    return (xrow8, xT, idx16, dstl, what, w_hat,
            NU, meta, ITOT, JCOLTOT)


def _fold_weights(Wcheb, bcheb, Wconv, bconv):
    Wcheb = np.asarray(Wcheb, np.float32)
    bcheb = np.asarray(bcheb, np.float32)
    Wconv = np.asarray(Wconv, np.float32)
    bconv = np.asarray(bconv, np.float32)
    pairs = []
    for go in range(3):
        for gi in range(max(0, go - 1), min(3, go + 2)):
            for path in range(2):
                pairs.append((path, gi, go))
    mats = np.zeros((len(pairs), 128, 128), np.float32)
    for pi, (path, gi, go) in enumerate(pairs):
        for wo in range(4 * go, 4 * go + 4):
            for k in range(3):
                wi = wo + k - 1
                if not (4 * gi <= wi < 4 * gi + 4) or not (0 <= wi < W):
                    continue
                Cmat = Wcheb[wi, path] @ Wconv[:, :, k].T  # [ci, co]
                r0 = 32 * (wi - 4 * gi)
                c0 = 32 * (wo - 4 * go)
                mats[pi, r0:r0 + 32, c0:c0 + 32] = Cmat
    mats_sb = np.ascontiguousarray(
        mats.transpose(1, 0, 2).reshape(128, -1)).astype(np.float16)
    bias = np.zeros((12, 32), np.float32)
    for wo in range(12):
        bias[wo] = bconv.copy()
        for k in range(3):
            wi = wo + k - 1
            if 0 <= wi < W:
                bias[wo] += bcheb[wi] @ Wconv[:, :, k].T
    bias_sb = bias.reshape(3, 128).T.copy()  # [128, 3]
    return mats_sb, bias_sb, pairs


def _build_program(NU, meta, ITOT, JCOLTOT, n_pairs):
    import concourse.bacc as bacc
    import concourse.tile as tile
    from concourse import mybir

    nc = bacc.Bacc("TRN2", target_bir_lowering=False, debug=False,
                   num_devices=NCORES)
    f16, f32, i16 = mybir.dt.float16, mybir.dt.float32, mybir.dt.int16
    f8 = mybir.dt.float8e4
    xrow8 = nc.dram_tensor("xrow8", [NPAD, GELEM], f8, kind="ExternalInput")
    xTd = nc.dram_tensor("xT", [128, 3, SLOTS * P], f16, kind="ExternalInput")
    idxd = nc.dram_tensor("idx16", [128, ITOT], i16, kind="ExternalInput")
    dstld = nc.dram_tensor("dstl", [128, JCOLTOT], f16, kind="ExternalInput")
    whatd = nc.dram_tensor("what", [128, JCOLTOT], f16, kind="ExternalInput")
    matsd = nc.dram_tensor("mats", [128, n_pairs * 128], f16, kind="ExternalInput")
    biasd = nc.dram_tensor("biasd", [128, 3], f32, kind="ExternalInput")
    iotad = nc.dram_tensor("iota", [128, 128], f16, kind="ExternalInput")
    out_pc = nc.dram_tensor("out_pc", [128, 3, SLOTS * P], f16, kind="ExternalOutput")

    pairs_by_go = [[], [], []]
    pi = 0
    for go in range(3):
        for gi in range(max(0, go - 1), min(3, go + 2)):
            for path in range(2):
                pairs_by_go[go].append((pi, gi, path))
                pi += 1

    JSMAX = max(sum(m["jc"] for m in meta[i]) for i in range(SLOTS))
    # per-slot (call, s) one-hot column ranges
    WMX = 0
    WSMAX = 0
    for i in range(SLOTS):
        wtot = 0
        for m in meta[i]:
            for s in range(m["slo"], m["shi"]):
                nu = int(NU[i, m["h"], s])
                if nu == 0:
                    continue
                o0 = int(m["offs"][s - m["slo"]])
                wn = (o0 + nu - 1) // 128 - o0 // 128 + 1
                wtot += wn
                WSMAX = max(WSMAX, wn)
        WMX = max(WMX, wtot)

    with tile.TileContext(nc) as tc:
        with tc.tile_pool(name="const", bufs=1) as cp, \
             tc.tile_pool(name="xgp", bufs=2) as xgp, \
             tc.tile_pool(name="wmp", bufs=2) as wmp, \
             tc.tile_pool(name="eqp", bufs=2) as eqp, \
             tc.tile_pool(name="t1p", bufs=2) as t1p, \
             tc.tile_pool(name="tlp", bufs=2) as tlp, \
             tc.tile_pool(name="stp", bufs=2) as stp, \
             tc.tile_pool(name="pst1", bufs=2, space="PSUM") as pst1, \
             tc.tile_pool(name="psy", bufs=2, space="PSUM") as psy:
            # small idx slice for slots 0-1 first so their gathers start
            # immediately; the big loads all queue behind them.
            S01 = int(meta[2][0]["io"])
            idx0_t = cp.tile([128, S01], i16)
            nc.sync.dma_start(out=idx0_t[:], in_=idxd.ap()[:, :S01])
            idx_t = cp.tile([128, ITOT], i16)
            nc.sync.dma_start(out=idx_t[:], in_=idxd.ap())
            zero3 = cp.tile([128, 3, 128], f8)
            nc.vector.memset(zero3[:], 0.0)
            mats_t = cp.tile([128, n_pairs * 128], f16)
            bias_t = cp.tile([128, 3], f32)
            iota_t = cp.tile([128, 128], f16)
            dm_t = cp.tile([128, JCOLTOT], f16)
            wh_t = cp.tile([128, JCOLTOT], f16)
            xt_t = cp.tile([128, 3, SLOTS * P], f16)

            # 7-slot output batches, with small final batches so the last
            # slots' compute tail overlaps earlier writes
            bstart = {0: 7, 7: 7, 14: 7, 21: 7, 28: 7, 35: 7,
                      42: 3, 45: 2, 47: 1, 48: 1}
            stage_t = None
            bs, bw = 0, 7
            for i in range(SLOTS):
                xg_t = xgp.tile([128, JSMAX, GELEM], f8, tag="xg")
                for m in meta[i]:
                    it = idx0_t if i < 2 else idx_t
                    nc.gpsimd.dma_gather(
                        xg_t[:, m["ch"]:m["ch"] + m["jc"], :],
                        xrow8.ap()[m["h"] * HALF:(m["h"] + 1) * HALF, :],
                        it[:, m["io"]:m["io"] + (m["nexact"] + 15) // 16],
                        m["nexact"], m["nexact"], GELEM,
                        single_packet=False)
                if i == 0:
                    nc.sync.dma_start(out=dm_t[:], in_=dstld.ap())
                    nc.sync.dma_start(out=wh_t[:], in_=whatd.ap())
                    nc.sync.dma_start(out=iota_t[:], in_=iotad.ap())
                    nc.sync.dma_start(out=mats_t[:], in_=matsd.ap())
                    nc.sync.dma_start(out=bias_t[:], in_=biasd.ap())
                    nc.sync.dma_start(out=xt_t[:], in_=xTd.ap())
                if i in bstart:
                    bs, bw = i, bstart[i]
                    stage_t = stp.tile([128, 3, 7 * P], f16, tag="st")

                # one-hot * w_hat masks (fp16 so w_hat stays exact; the
                # message matmul mixes fp16 moving x fp8 stationary)
                wm_t = wmp.tile([128, WMX, 32], f16, tag="wm")
                woff = 0
                mm_by_col = {}  # (xg col, rows) -> list of (wm col, s)
                for m in meta[i]:
                    lastc = m["ch"] + m["jc"] - 1
                    rem = m["nexact"] % 128
                    for s in range(m["slo"], m["shi"]):
                        nu = int(NU[i, m["h"], s])
                        if nu == 0:
                            continue
                        o0 = int(m["offs"][s - m["slo"]])
                        c0, c1 = o0 // 128, (o0 + nu - 1) // 128
                        wn = c1 - c0 + 1
                        a = int(m["co"]) + c0
                        eq_t = eqp.tile([128, WSMAX, 32], f16, tag="eq")
                        nc.vector.tensor_tensor(
                            out=eq_t[:, :wn, :],
                            in0=dm_t[:, a:a + wn].unsqueeze(2).to_broadcast([128, wn, 32]),
                            in1=iota_t[:, 32 * s:32 * s + 32].unsqueeze(1).to_broadcast([128, wn, 32]),
                            op=mybir.AluOpType.is_equal)
                        nc.vector.tensor_tensor(
                            out=wm_t[:, woff:woff + wn, :],
                            in0=eq_t[:, :wn, :],
                            in1=wh_t[:, a:a + wn].unsqueeze(2).to_broadcast([128, wn, 32]),
                            op=mybir.AluOpType.mult)
                        for q in range(wn):
                            xc = m["ch"] + c0 + q
                            pr = rem if (xc == lastc and rem) else 128
                            mm_by_col.setdefault((xc, pr), []).append((woff + q, s))
                        woff += wn

                # message aggregation, channel-major: t1T[ch, dst] in PSUM.
                # call-major / column-major order so matmuls chase gathers.
                pst = pst1.tile([128, 3, 128], f32, space="PSUM", tag="t1")
                nc.tensor.matmul(out=pst[:], lhsT=zero3[:, 0, :], rhs=zero3[:],
                                 start=True, stop=False, skip_group_check=True)
                cols = sorted(mm_by_col.items())
                total = 3 * sum(len(v) for _, v in cols)
                k = 0
                for (xcol, pr), ws in cols:
                    for b in range(3):
                        for (wcol, s) in ws:
                            k += 1
                            nc.tensor.matmul(
                                out=pst[:, b, 32 * s:32 * s + 32],
                                lhsT=xg_t[0:pr, xcol, b * 128:(b + 1) * 128],
                                rhs=wm_t[0:pr, wcol, :],
                                start=False, stop=(k == total),
                                skip_group_check=True)

                t1sb = t1p.tile([128, 3, 128], f16, tag="t1sb")
                nc.scalar.mul(out=t1sb[:], in_=pst[:], mul=DS)

                # cheb + temporal-conv fold (channel-major y)
                yps = psy.tile([128, 3, 128], f32, space="PSUM", tag="y")
                for go in range(3):
                    plist = pairs_by_go[go]
                    for n_, (pi_, gi, path) in enumerate(plist):
                        rhs = (xt_t[:, gi, i * P:(i + 1) * P] if path == 0
                               else t1sb[:, gi, :])
                        nc.tensor.matmul(
                            out=yps[:, go, :],
                            lhsT=mats_t[:, pi_ * 128:(pi_ + 1) * 128],
                            rhs=rhs,
                            start=(n_ == 0), stop=(n_ == len(plist) - 1),
                            skip_group_check=True)
                    ysl = stage_t[:, go, (i - bs) * P:(i - bs + 1) * P]
                    nc.scalar.activation(out=ysl, in_=yps[:, go, :],
                                         func=mybir.ActivationFunctionType.Identity,
                                         bias=bias_t[:, go:go + 1], scale=1.0)
                    tl = tlp.tile([128, 128], f16, tag="tl")
                    nc.vector.tensor_scalar_mul(out=tl[:], in0=ysl, scalar1=0.01)
                    nc.vector.tensor_tensor(out=ysl, in0=ysl, in1=tl[:],
                                            op=mybir.AluOpType.max)

                if i == bs + bw - 1:
                    nc.sync.dma_start(
                        out=out_pc.ap()[:, :, bs * P:(bs + bw) * P],
                        in_=stage_t[:, :, 0:bw * P])

    nc.compile()
    return nc


def kernel(x, A, Ew, Wcheb, bcheb, Wconv, bconv, batch_size=1):
    from concourse.bass_utils import run_bass_kernel_spmd

    (xrow8, xT, idx16, dstl, what, w_hat,
     NU, meta, ITOT, JCOLTOT) = _host_prep(x, A, Ew)
    mats_sb, bias_sb, pairs = _fold_weights(Wcheb, bcheb, Wconv, bconv)

    key = (ITOT, JCOLTOT, tuple(NU.reshape(-1).tolist()))
    if key not in _cache:
        _cache[key] = _build_program(NU, meta, ITOT, JCOLTOT, len(pairs))
    nc = _cache[key]

    iota_np = np.tile(np.arange(128, dtype=np.float16)[None, :], (128, 1))
    in_maps = []
    for c in range(NCORES):
        in_maps.append(dict(
            xrow8=xrow8,
            xT=np.ascontiguousarray(xT[:, :, c * SLOTS * P:(c + 1) * SLOTS * P]),
            idx16=idx16[c], dstl=dstl[c], what=what[c],
            mats=mats_sb, biasd=bias_sb, iota=iota_np))
    res = run_bass_kernel_spmd(nc, in_maps, core_ids=list(range(NCORES)))
    # out_pc[c]: [128, 3, 6272] channel-major fp16 -> [50000, 12, 32] f32
    cols = [np.asarray(res.results[c]["out_pc"], np.float16).reshape(128, 3, SLOTS * P)
            for c in range(NCORES)]
    full = np.concatenate(cols, axis=2)                  # [128, 3, 50176]
    full = full.transpose(1, 0, 2).reshape(WC, NPAD)     # [384, 50176]
    y = np.ascontiguousarray(full[:, :N].T).astype(np.float32)  # [N, 384]
    return y.reshape(N, W, C)


# revision 26
# speedup vs baseline: 1.0072x; 1.0038x over previous
"""ChebConv (K=2) + temporal Conv1d GNN kernel for 8 Trainium2 NeuronCores.

Strategy (data-parallel over destination nodes, channel-major on chip):
  - Node axis padded to 50176 = 392 blocks of 128; core c owns blocks
    [49c, 49c+49).
  - Host precomputes w_hat (edge weights of -D^-1/2 A D^-1/2), quantizes x
    to fp8-e4m3 rows padded to 512B (descriptor-efficient gathers), and
    sorts the edge list by (dst block, src half, dst subblock-of-32) with
    16-aligned group sizes shared across cores (max over cores).
  - Per block the device gathers fp8 source rows with SWDGE dma_gather,
    builds 32-wide one-hot*w_hat fp8 masks on DVE, and aggregates messages
    with TensorE matmuls that keep the result CHANNEL-major (x rows are the
    stationary operand), so no on-chip transposes are needed anywhere.
  - Chebyshev combine + temporal conv collapse into 14 dense 128x128
    matmuls per block with host-prefolded fp16 weights (x^T streamed from
    HBM); LeakyReLU finishes on-chip; fp16 channel-major output is
    de-transposed on the host.
"""

import numpy as np
import ml_dtypes

N = 50000
E = 1600000
W = 12
C = 32
WC = W * C            # 384
NCORES = 8
P = 128
NPAD = 50176          # 392 * 128
NB = NPAD // P        # 392
SLOTS = NB // NCORES  # 49
HALF = NPAD // 2      # 25088
GELEM = 512           # fp8 row bytes (384 data + 128 pad)
XS = 8.0              # x fp8 scale
WS = 64.0             # w_hat fp8 scale
DS = 1.0 / (XS * WS)
F8NP = ml_dtypes.float8_e4m3

_cache = {}


def _host_prep(x, A, Ew):
    src = np.asarray(A[0], np.int64)
    dst = np.asarray(A[1], np.int64)
    Ew = np.asarray(Ew, np.float32)

    deg = np.bincount(dst, weights=Ew.astype(np.float64), minlength=N).astype(np.float32)
    dinv = np.where(deg > 0, 1.0 / np.sqrt(np.maximum(deg, 1e-12)), 0.0).astype(np.float32)
    w_hat = (-dinv[src] * Ew * dinv[dst]).astype(np.float32)

    xn = np.asarray(x, np.float32).transpose(1, 0, 2).reshape(N, WC)
    xrow8 = np.zeros((NPAD, GELEM), F8NP)
    xrow8[:N, :WC] = np.clip(xn * XS, -224.0, 224.0).astype(F8NP)
    xpad = np.zeros((NPAD, WC), np.float16)
    xpad[:N] = xn
    # channel-major x for the cheb-fold matmuls: [128, 3, NPAD]
    xT = np.ascontiguousarray(xpad.T.reshape(3, 128, NPAD).transpose(1, 0, 2))

    blk = dst >> 7
    hh = (src >= HALF).astype(np.int64)
    sb = (dst >> 5) & 3
    gid = (blk * 2 + hh) * 4 + sb
    order = np.argsort(gid, kind="stable")
    loc = (src[order] - hh[order] * HALF).astype(np.int16)
    dl = (dst[order] & 127).astype(np.float16)
    wv = np.clip(w_hat[order] * WS, -224.0, 224.0).astype(np.float16)

    counts = np.bincount(gid, minlength=NB * 8)
    gstart = np.zeros(NB * 8 + 1, np.int64)
    np.cumsum(counts, out=gstart[1:])
    ccore = counts.reshape(NCORES, SLOTS, 2, 4)
    NU = ccore.max(axis=0).astype(np.int64)                       # [49, 2, 4]

    # per-slot gather-call list; the last slots use split calls so their
    # message matmuls can chase partial gathers (shorter pipeline tail)
    CALLS = []
    for i in range(SLOTS):
        if i >= SLOTS - 2:
            CALLS.append([(0, 0, 2), (0, 2, 4), (1, 0, 2), (1, 2, 4)])
        else:
            CALLS.append([(0, 0, 4), (1, 0, 4)])

    # per-call static layout: exact index count, 16-aligned idx span,
    # column count / offsets, and per-s offsets within the call
    meta = []   # per slot: list of dicts
    io = co = 0
    for i in range(SLOTS):
        mslot = []
        ch = 0
        for (h, slo, shi) in CALLS[i]:
            offs = np.zeros(5, np.int64)
            np.cumsum(NU[i, h, slo:shi], out=offs[1:shi - slo + 1])
            nexact = int(offs[shi - slo])
            n16 = (nexact + 15) // 16 * 16
            jc = -(-n16 // 128)
            mslot.append(dict(h=h, slo=slo, shi=shi, offs=offs,
                              nexact=nexact, n16=n16, jc=jc,
                              io=io, co=co, ch=ch))
            io += n16 // 16
            co += jc
            ch += jc
        meta.append(mslot)
    ITOT = io
    JCOLTOT = co

    idx16 = np.zeros((NCORES, 128, ITOT), np.int16)
    dstl = np.full((NCORES, 128, JCOLTOT), 255.0, np.float16)
    what = np.zeros((NCORES, 128, JCOLTOT), np.float16)

    for c in range(NCORES):
        for i in range(SLOTS):
            for m in meta[i]:
                n16, jc = m["n16"], m["jc"]
                V = np.zeros(n16, np.int16)
                D = np.full(jc * 128, 255.0, np.float16)
                Wv = np.zeros(jc * 128, np.float16)
                for s in range(m["slo"], m["shi"]):
                    g = (((c * SLOTS + i) * 2 + m["h"]) * 4 + s)
                    n = int(counts[g])
                    sl = slice(int(gstart[g]), int(gstart[g]) + n)
                    o = int(m["offs"][s - m["slo"]])
                    V[o:o + n] = loc[sl]
                    D[o:o + n] = dl[sl]
                    Wv[o:o + n] = wv[sl]
                idx16[c, :, m["io"]:m["io"] + n16 // 16] = \
                    np.tile(V.reshape(-1, 16).T, (8, 1))
                dstl[c, :, m["co"]:m["co"] + jc] = D.reshape(jc, 128).T
                what[c, :, m["co"]:m["co"] + jc] = Wv.reshape(jc, 128).T

    return (xrow8, xT, idx16, dstl, what, w_hat,
            NU, meta, ITOT, JCOLTOT)


def _fold_weights(Wcheb, bcheb, Wconv, bconv):
    Wcheb = np.asarray(Wcheb, np.float32)
    bcheb = np.asarray(bcheb, np.float32)
    Wconv = np.asarray(Wconv, np.float32)
    bconv = np.asarray(bconv, np.float32)
    pairs = []
    for go in range(3):
        for gi in range(max(0, go - 1), min(3, go + 2)):
            for path in range(2):
                pairs.append((path, gi, go))
    mats = np.zeros((len(pairs), 128, 128), np.float32)
    for pi, (path, gi, go) in enumerate(pairs):
        for wo in range(4 * go, 4 * go + 4):
            for k in range(3):
                wi = wo + k - 1
                if not (4 * gi <= wi < 4 * gi + 4) or not (0 <= wi < W):
                    continue
                Cmat = Wcheb[wi, path] @ Wconv[:, :, k].T  # [ci, co]
                r0 = 32 * (wi - 4 * gi)
                c0 = 32 * (wo - 4 * go)
                mats[pi, r0:r0 + 32, c0:c0 + 32] = Cmat
    mats_sb = np.ascontiguousarray(
        mats.transpose(1, 0, 2).reshape(128, -1)).astype(np.float16)
    bias = np.zeros((12, 32), np.float32)
    for wo in range(12):
        bias[wo] = bconv.copy()
        for k in range(3):
            wi = wo + k - 1
            if 0 <= wi < W:
                bias[wo] += bcheb[wi] @ Wconv[:, :, k].T
    bias_sb = bias.reshape(3, 128).T.copy()  # [128, 3]
    return mats_sb, bias_sb, pairs


def _build_program(NU, meta, ITOT, JCOLTOT, n_pairs):
    import concourse.bacc as bacc
    import concourse.tile as tile
    from concourse import mybir

    nc = bacc.Bacc("TRN2", target_bir_lowering=False, debug=False,
                   num_devices=NCORES)
    f16, f32, i16 = mybir.dt.float16, mybir.dt.float32, mybir.dt.int16
    f8 = mybir.dt.float8e4
    xrow8 = nc.dram_tensor("xrow8", [NPAD, GELEM], f8, kind="ExternalInput")
    xTd = nc.dram_tensor("xT", [128, 3, SLOTS * P], f16, kind="ExternalInput")
    idxd = nc.dram_tensor("idx16", [128, ITOT], i16, kind="ExternalInput")
    dstld = nc.dram_tensor("dstl", [128, JCOLTOT], f16, kind="ExternalInput")
    whatd = nc.dram_tensor("what", [128, JCOLTOT], f16, kind="ExternalInput")
    matsd = nc.dram_tensor("mats", [128, n_pairs * 128], f16, kind="ExternalInput")
    biasd = nc.dram_tensor("biasd", [128, 3], f32, kind="ExternalInput")
    iotad = nc.dram_tensor("iota", [128, 128], f16, kind="ExternalInput")
    out_pc = nc.dram_tensor("out_pc", [128, 3, SLOTS * P], f16, kind="ExternalOutput")

    pairs_by_go = [[], [], []]
    pi = 0
    for go in range(3):
        for gi in range(max(0, go - 1), min(3, go + 2)):
            for path in range(2):
                pairs_by_go[go].append((pi, gi, path))
                pi += 1

    NCALLS = max(len(meta[i]) for i in range(SLOTS))
    JCM = [max(meta[i][k]["jc"] for i in range(SLOTS) if len(meta[i]) > k)
           for k in range(NCALLS)]
    # per-slot (call, s) one-hot column ranges
    WMX = 0
    WSMAX = 0
    for i in range(SLOTS):
        wtot = 0
        for m in meta[i]:
            for s in range(m["slo"], m["shi"]):
                nu = int(NU[i, m["h"], s])
                if nu == 0:
                    continue
                o0 = int(m["offs"][s - m["slo"]])
                wn = (o0 + nu - 1) // 128 - o0 // 128 + 1
                wtot += wn
                WSMAX = max(WSMAX, wn)
        WMX = max(WMX, wtot)

    with tile.TileContext(nc) as tc:
        with tc.tile_pool(name="const", bufs=1) as cp, \
             tc.tile_pool(name="xgp", bufs=2) as xgp, \
             tc.tile_pool(name="wmp", bufs=2) as wmp, \
             tc.tile_pool(name="eqp", bufs=2) as eqp, \
             tc.tile_pool(name="t1p", bufs=2) as t1p, \
             tc.tile_pool(name="tlp", bufs=2) as tlp, \
             tc.tile_pool(name="stp", bufs=2) as stp, \
             tc.tile_pool(name="pst1", bufs=2, space="PSUM") as pst1, \
             tc.tile_pool(name="psy", bufs=2, space="PSUM") as psy:
            # small idx slice for slots 0-1 first so their gathers start
            # immediately; the big loads all queue behind them.
            S01 = int(meta[2][0]["io"])
            idx0_t = cp.tile([128, S01], i16)
            nc.sync.dma_start(out=idx0_t[:], in_=idxd.ap()[:, :S01])
            idx_t = cp.tile([128, ITOT], i16)
            nc.sync.dma_start(out=idx_t[:], in_=idxd.ap())
            zero3 = cp.tile([128, 3, 128], f8)
            nc.vector.memset(zero3[:], 0.0)
            mats_t = cp.tile([128, n_pairs * 128], f16)
            bias_t = cp.tile([128, 3], f32)
            iota_t = cp.tile([128, 128], f16)
            dm_t = cp.tile([128, JCOLTOT], f16)
            wh_t = cp.tile([128, JCOLTOT], f16)
            xt_t = cp.tile([128, 3, SLOTS * P], f16)

            # 7-slot output batches, with small final batches so the last
            # slots' compute tail overlaps earlier writes
            bstart = {0: 7, 7: 7, 14: 7, 21: 7, 28: 7, 35: 7,
                      42: 3, 45: 2, 47: 1, 48: 1}
            stage_t = None
            bs, bw = 0, 7
            for i in range(SLOTS):
                # one gather tile per call so matmuls only wait on their
                # own call's gather, not the whole slot's
                xgs = []
                for k, m in enumerate(meta[i]):
                    xg_t = xgp.tile([128, JCM[k], GELEM], f8, tag=f"xg{k}")
                    xgs.append(xg_t)
                    it = idx0_t if i < 2 else idx_t
                    nc.gpsimd.dma_gather(
                        xg_t[:, 0:m["jc"], :],
                        xrow8.ap()[m["h"] * HALF:(m["h"] + 1) * HALF, :],
                        it[:, m["io"]:m["io"] + (m["nexact"] + 15) // 16],
                        m["nexact"], m["nexact"], GELEM,
                        single_packet=False)
                if i == 0:
                    nc.sync.dma_start(out=dm_t[:], in_=dstld.ap())
                    nc.sync.dma_start(out=wh_t[:], in_=whatd.ap())
                    nc.sync.dma_start(out=iota_t[:], in_=iotad.ap())
                    nc.sync.dma_start(out=mats_t[:], in_=matsd.ap())
                    nc.sync.dma_start(out=bias_t[:], in_=biasd.ap())
                    nc.sync.dma_start(out=xt_t[:], in_=xTd.ap())
                if i in bstart:
                    bs, bw = i, bstart[i]
                    stage_t = stp.tile([128, 3, 7 * P], f16, tag="st")

                # one-hot * w_hat masks (fp16 so w_hat stays exact; the
                # message matmul mixes fp16 moving x fp8 stationary)
                wm_t = wmp.tile([128, WMX, 32], f16, tag="wm")
                woff = 0
                mm_by_col = {}  # (call, xg col, rows) -> list of (wm col, s)
                for ci, m in enumerate(meta[i]):
                    lastc = m["jc"] - 1
                    rem = m["nexact"] % 128
                    for s in range(m["slo"], m["shi"]):
                        nu = int(NU[i, m["h"], s])
                        if nu == 0:
                            continue
                        o0 = int(m["offs"][s - m["slo"]])
                        c0, c1 = o0 // 128, (o0 + nu - 1) // 128
                        wn = c1 - c0 + 1
                        a = int(m["co"]) + c0
                        eq_t = eqp.tile([128, WSMAX, 32], f16, tag="eq")
                        nc.vector.tensor_tensor(
                            out=eq_t[:, :wn, :],
                            in0=dm_t[:, a:a + wn].unsqueeze(2).to_broadcast([128, wn, 32]),
                            in1=iota_t[:, 32 * s:32 * s + 32].unsqueeze(1).to_broadcast([128, wn, 32]),
                            op=mybir.AluOpType.is_equal)
                        nc.vector.tensor_tensor(
                            out=wm_t[:, woff:woff + wn, :],
                            in0=eq_t[:, :wn, :],
                            in1=wh_t[:, a:a + wn].unsqueeze(2).to_broadcast([128, wn, 32]),
                            op=mybir.AluOpType.mult)
                        for q in range(wn):
                            xc = c0 + q
                            pr = rem if (xc == lastc and rem) else 128
                            mm_by_col.setdefault((ci, xc, pr), []).append((woff + q, s))
                        woff += wn

                # message aggregation, channel-major: t1T[ch, dst] in PSUM.
                # call-major / column-major order so matmuls chase gathers.
                pst = pst1.tile([128, 3, 128], f32, space="PSUM", tag="t1")
                nc.tensor.matmul(out=pst[:], lhsT=zero3[:, 0, :], rhs=zero3[:],
                                 start=True, stop=False, skip_group_check=True)
                cols = sorted(mm_by_col.items())
                total = 3 * sum(len(v) for _, v in cols)
                k = 0
                for (ci, xcol, pr), ws in cols:
                    for b in range(3):
                        for (wcol, s) in ws:
                            k += 1
                            nc.tensor.matmul(
                                out=pst[:, b, 32 * s:32 * s + 32],
                                lhsT=xgs[ci][0:pr, xcol, b * 128:(b + 1) * 128],
                                rhs=wm_t[0:pr, wcol, :],
                                start=False, stop=(k == total),
                                skip_group_check=True)

                t1sb = t1p.tile([128, 3, 128], f16, tag="t1sb")
                nc.scalar.mul(out=t1sb[:], in_=pst[:], mul=DS)

                # cheb + temporal-conv fold (channel-major y); separate psum
                # tile per go so act(go) doesn't serialize go+1's matmuls
                for go in range(3):
                    yps = psy.tile([128, 128], f32, space="PSUM", tag=f"y{go}")
                    plist = pairs_by_go[go]
                    for n_, (pi_, gi, path) in enumerate(plist):
                        rhs = (xt_t[:, gi, i * P:(i + 1) * P] if path == 0
                               else t1sb[:, gi, :])
                        nc.tensor.matmul(
                            out=yps[:],
                            lhsT=mats_t[:, pi_ * 128:(pi_ + 1) * 128],
                            rhs=rhs,
                            start=(n_ == 0), stop=(n_ == len(plist) - 1),
                            skip_group_check=True)
                    ysl = stage_t[:, go, (i - bs) * P:(i - bs + 1) * P]
                    nc.scalar.activation(out=ysl, in_=yps[:],
                                         func=mybir.ActivationFunctionType.Identity,
                                         bias=bias_t[:, go:go + 1], scale=1.0)
                    tl = tlp.tile([128, 128], f16, tag="tl")
                    nc.vector.tensor_scalar_mul(out=tl[:], in0=ysl, scalar1=0.01)
                    nc.vector.tensor_tensor(out=ysl, in0=ysl, in1=tl[:],
                                            op=mybir.AluOpType.max)

                if i == bs + bw - 1:
                    nc.sync.dma_start(
                        out=out_pc.ap()[:, :, bs * P:(bs + bw) * P],
                        in_=stage_t[:, :, 0:bw * P])

    nc.compile()
    return nc


def kernel(x, A, Ew, Wcheb, bcheb, Wconv, bconv, batch_size=1):
    from concourse.bass_utils import run_bass_kernel_spmd

    (xrow8, xT, idx16, dstl, what, w_hat,
     NU, meta, ITOT, JCOLTOT) = _host_prep(x, A, Ew)
    mats_sb, bias_sb, pairs = _fold_weights(Wcheb, bcheb, Wconv, bconv)

    key = (ITOT, JCOLTOT, tuple(NU.reshape(-1).tolist()))
    if key not in _cache:
        _cache[key] = _build_program(NU, meta, ITOT, JCOLTOT, len(pairs))
    nc = _cache[key]

    iota_np = np.tile(np.arange(128, dtype=np.float16)[None, :], (128, 1))
    in_maps = []
    for c in range(NCORES):
        in_maps.append(dict(
            xrow8=xrow8,
            xT=np.ascontiguousarray(xT[:, :, c * SLOTS * P:(c + 1) * SLOTS * P]),
            idx16=idx16[c], dstl=dstl[c], what=what[c],
            mats=mats_sb, biasd=bias_sb, iota=iota_np))
    res = run_bass_kernel_spmd(nc, in_maps, core_ids=list(range(NCORES)))
    # out_pc[c]: [128, 3, 6272] channel-major fp16 -> [50000, 12, 32] f32
    cols = [np.asarray(res.results[c]["out_pc"], np.float16).reshape(128, 3, SLOTS * P)
            for c in range(NCORES)]
    full = np.concatenate(cols, axis=2)                  # [128, 3, 50176]
    full = full.transpose(1, 0, 2).reshape(WC, NPAD)     # [384, 50176]
    y = np.ascontiguousarray(full[:, :N].T).astype(np.float32)  # [N, 384]
    return y.reshape(N, W, C)


# revision 32
# speedup vs baseline: 1.0612x; 1.0536x over previous
"""ChebConv (K=2) + temporal Conv1d GNN kernel for 8 Trainium2 NeuronCores.

Strategy (data-parallel over destination nodes, channel-major on chip):
  - Node axis padded to 50176 = 392 blocks of 128; core c owns blocks
    [49c, 49c+49).
  - Host precomputes w_hat (edge weights of -D^-1/2 A D^-1/2), quantizes x
    to fp8-e4m3 rows padded to 512B (descriptor-efficient gathers), and
    sorts the edge list by (dst block, src half, dst subblock-of-32) with
    16-aligned group sizes shared across cores (max over cores).
  - Per block the device gathers fp8 source rows with SWDGE dma_gather,
    builds 32-wide one-hot*w_hat fp8 masks on DVE, and aggregates messages
    with TensorE matmuls that keep the result CHANNEL-major (x rows are the
    stationary operand), so no on-chip transposes are needed anywhere.
  - Chebyshev combine + temporal conv collapse into 14 dense 128x128
    matmuls per block with host-prefolded fp16 weights (x^T streamed from
    HBM); LeakyReLU finishes on-chip; fp16 channel-major output is
    de-transposed on the host.
"""

import numpy as np
import ml_dtypes

N = 50000
E = 1600000
W = 12
C = 32
WC = W * C            # 384
NCORES = 8
P = 128
NPAD = 50176          # 392 * 128
NB = NPAD // P        # 392
SLOTS = NB // NCORES  # 49
WIN = 32768           # gather window rows (int16 index range)
B1 = NPAD - WIN       # 17408: second window base; [B1, WIN) is flexible
GELEM = 512           # fp8 row bytes (384 data + 128 pad)
XS = 8.0              # x fp8 scale
WS = 64.0             # w_hat fp8 scale
DS = 1.0 / (XS * WS)
F8NP = ml_dtypes.float8_e4m3

_cache = {}


def _host_prep(x, A, Ew):
    src = np.asarray(A[0], np.int64)
    dst = np.asarray(A[1], np.int64)
    Ew = np.asarray(Ew, np.float32)

    deg = np.bincount(dst, weights=Ew.astype(np.float64), minlength=N).astype(np.float32)
    dinv = np.where(deg > 0, 1.0 / np.sqrt(np.maximum(deg, 1e-12)), 0.0).astype(np.float32)
    w_hat = (-dinv[src] * Ew * dinv[dst]).astype(np.float32)

    # --- node permutation: snake-assign nodes by in-degree so every
    # 32-node destination subgroup has near-equal edge count (the shared
    # program pads each group to the max over cores)
    degc = np.zeros(NPAD, np.int64)
    degc[:N] = np.bincount(dst, minlength=N)
    byorder = np.argsort(-degc, kind="stable").reshape(32, NB * 4)
    byorder[1::2] = byorder[1::2, ::-1]
    g_idx = np.arange(NB * 4)
    tgt = (g_idx[None, :] // 4) * 128 + (g_idx[None, :] % 4) * 32 \
        + np.arange(32)[:, None]
    newid = np.empty(NPAD, np.int64)
    newid[byorder] = tgt
    src2 = newid[src]
    dst2 = newid[dst]

    xn = np.asarray(x, np.float32).transpose(1, 0, 2).reshape(N, WC)
    xrow8 = np.zeros((NPAD, GELEM), F8NP)
    xrow8[newid[:N], :WC] = np.clip(xn * XS, -224.0, 224.0).astype(F8NP)
    xpad = np.zeros((NPAD, WC), np.float16)
    xpad[newid[:N]] = xn
    # channel-major x for the cheb-fold matmuls: [128, 3, NPAD]
    xT = np.ascontiguousarray(xpad.T.reshape(3, 128, NPAD).transpose(1, 0, 2))

    blk = dst2 >> 7
    sb = (dst2 >> 5) & 3
    # --- overlapping gather windows: srcs in [B1, WIN) may use either
    # window; split every (group) exactly in half across the two calls
    grp = blk * 4 + sb
    cls = np.where(src2 < B1, 0, np.where(src2 < WIN, 1, 2))
    bins = grp * 4 + cls
    eorder = np.argsort(bins, kind="stable")
    bstart_ = np.zeros(NB * 16 + 1, np.int64)
    np.cumsum(np.bincount(bins, minlength=NB * 16), out=bstart_[1:])
    cnt_gc = np.bincount(bins, minlength=NB * 16).reshape(NB * 4, 4)
    tgc = cnt_gc[:, :3].sum(1)
    n0 = (tgc + 1) // 2
    flex0 = np.clip(n0 - cnt_gc[:, 0], 0, cnt_gc[:, 1])
    rank = np.arange(E) - bstart_[bins[eorder]]
    cls_s = cls[eorder]
    grp_s = grp[eorder]
    h_s = np.where(cls_s == 0, 0,
                   np.where(cls_s == 2, 1,
                            (rank >= flex0[grp_s]).astype(np.int64)))
    hh = np.empty(E, np.int64)
    hh[eorder] = h_s

    gid = (blk * 2 + hh) * 4 + sb
    order = np.argsort(gid, kind="stable")
    loc = (src2[order] - hh[order] * B1).astype(np.int16)
    dl = (dst2[order] & 127).astype(np.float16)
    wv = np.clip(w_hat[order] * WS, -224.0, 224.0).astype(np.float16)

    counts = np.bincount(gid, minlength=NB * 8)
    gstart = np.zeros(NB * 8 + 1, np.int64)
    np.cumsum(counts, out=gstart[1:])
    ccore = counts.reshape(NCORES, SLOTS, 2, 4)
    NU = ccore.max(axis=0).astype(np.int64)                       # [49, 2, 4]

    # per-slot gather-call list; the last slots use split calls so their
    # message matmuls can chase partial gathers (shorter pipeline tail)
    CALLS = []
    for i in range(SLOTS):
        if i >= SLOTS - 2:
            CALLS.append([(0, 0, 2), (0, 2, 4), (1, 0, 2), (1, 2, 4)])
        else:
            CALLS.append([(0, 0, 4), (1, 0, 4)])

    # per-call static layout: exact index count, 16-aligned idx span,
    # column count / offsets, and per-s offsets within the call
    meta = []   # per slot: list of dicts
    io = co = 0
    for i in range(SLOTS):
        mslot = []
        ch = 0
        for (h, slo, shi) in CALLS[i]:
            offs = np.zeros(5, np.int64)
            np.cumsum(NU[i, h, slo:shi], out=offs[1:shi - slo + 1])
            nexact = int(offs[shi - slo])
            n16 = (nexact + 15) // 16 * 16
            jc = -(-n16 // 128)
            mslot.append(dict(h=h, slo=slo, shi=shi, offs=offs,
                              nexact=nexact, n16=n16, jc=jc,
                              io=io, co=co, ch=ch))
            io += n16 // 16
            co += jc
            ch += jc
        meta.append(mslot)
    ITOT = io
    JCOLTOT = co

    idx16 = np.zeros((NCORES, 128, ITOT), np.int16)
    dstl = np.full((NCORES, 128, JCOLTOT), 255.0, np.float16)
    what = np.zeros((NCORES, 128, JCOLTOT), np.float16)

    for c in range(NCORES):
        for i in range(SLOTS):
            for m in meta[i]:
                n16, jc = m["n16"], m["jc"]
                V = np.zeros(n16, np.int16)
                D = np.full(jc * 128, 255.0, np.float16)
                Wv = np.zeros(jc * 128, np.float16)
                for s in range(m["slo"], m["shi"]):
                    g = (((c * SLOTS + i) * 2 + m["h"]) * 4 + s)
                    n = int(counts[g])
                    sl = slice(int(gstart[g]), int(gstart[g]) + n)
                    o = int(m["offs"][s - m["slo"]])
                    V[o:o + n] = loc[sl]
                    D[o:o + n] = dl[sl]
                    Wv[o:o + n] = wv[sl]
                idx16[c, :, m["io"]:m["io"] + n16 // 16] = \
                    np.tile(V.reshape(-1, 16).T, (8, 1))
                dstl[c, :, m["co"]:m["co"] + jc] = D.reshape(jc, 128).T
                what[c, :, m["co"]:m["co"] + jc] = Wv.reshape(jc, 128).T

    return (xrow8, xT, idx16, dstl, what, newid,
            NU, meta, ITOT, JCOLTOT)


def _fold_weights(Wcheb, bcheb, Wconv, bconv):
    Wcheb = np.asarray(Wcheb, np.float32)
    bcheb = np.asarray(bcheb, np.float32)
    Wconv = np.asarray(Wconv, np.float32)
    bconv = np.asarray(bconv, np.float32)
    pairs = []
    for go in range(3):
        for gi in range(max(0, go - 1), min(3, go + 2)):
            for path in range(2):
                pairs.append((path, gi, go))
    mats = np.zeros((len(pairs), 128, 128), np.float32)
    for pi, (path, gi, go) in enumerate(pairs):
        for wo in range(4 * go, 4 * go + 4):
            for k in range(3):
                wi = wo + k - 1
                if not (4 * gi <= wi < 4 * gi + 4) or not (0 <= wi < W):
                    continue
                Cmat = Wcheb[wi, path] @ Wconv[:, :, k].T  # [ci, co]
                r0 = 32 * (wi - 4 * gi)
                c0 = 32 * (wo - 4 * go)
                mats[pi, r0:r0 + 32, c0:c0 + 32] = Cmat
    mats_sb = np.ascontiguousarray(
        mats.transpose(1, 0, 2).reshape(128, -1)).astype(np.float16)
    bias = np.zeros((12, 32), np.float32)
    for wo in range(12):
        bias[wo] = bconv.copy()
        for k in range(3):
            wi = wo + k - 1
            if 0 <= wi < W:
                bias[wo] += bcheb[wi] @ Wconv[:, :, k].T
    bias_sb = bias.reshape(3, 128).T.copy()  # [128, 3]
    return mats_sb, bias_sb, pairs


def _build_program(NU, meta, ITOT, JCOLTOT, n_pairs):
    import concourse.bacc as bacc
    import concourse.tile as tile
    from concourse import mybir

    nc = bacc.Bacc("TRN2", target_bir_lowering=False, debug=False,
                   num_devices=NCORES)
    f16, f32, i16 = mybir.dt.float16, mybir.dt.float32, mybir.dt.int16
    f8 = mybir.dt.float8e4
    xrow8 = nc.dram_tensor("xrow8", [NPAD, GELEM], f8, kind="ExternalInput")
    xTd = nc.dram_tensor("xT", [128, 3, SLOTS * P], f16, kind="ExternalInput")
    idxd = nc.dram_tensor("idx16", [128, ITOT], i16, kind="ExternalInput")
    dstld = nc.dram_tensor("dstl", [128, JCOLTOT], f16, kind="ExternalInput")
    whatd = nc.dram_tensor("what", [128, JCOLTOT], f16, kind="ExternalInput")
    matsd = nc.dram_tensor("mats", [128, n_pairs * 128], f16, kind="ExternalInput")
    biasd = nc.dram_tensor("biasd", [128, 3], f32, kind="ExternalInput")
    iotad = nc.dram_tensor("iota", [128, 128], f16, kind="ExternalInput")
    out_pc = nc.dram_tensor("out_pc", [128, 3, SLOTS * P], f16, kind="ExternalOutput")

    pairs_by_go = [[], [], []]
    pi = 0
    for go in range(3):
        for gi in range(max(0, go - 1), min(3, go + 2)):
            for path in range(2):
                pairs_by_go[go].append((pi, gi, path))
                pi += 1

    NCALLS = max(len(meta[i]) for i in range(SLOTS))
    JCM = [max(meta[i][k]["jc"] for i in range(SLOTS) if len(meta[i]) > k)
           for k in range(NCALLS)]
    # per-slot (call, s) one-hot column ranges
    WMX = 0
    WSMAX = 0
    for i in range(SLOTS):
        wtot = 0
        for m in meta[i]:
            for s in range(m["slo"], m["shi"]):
                nu = int(NU[i, m["h"], s])
                if nu == 0:
                    continue
                o0 = int(m["offs"][s - m["slo"]])
                wn = (o0 + nu - 1) // 128 - o0 // 128 + 1
                wtot += wn
                WSMAX = max(WSMAX, wn)
        WMX = max(WMX, wtot)

    with tile.TileContext(nc) as tc:
        with tc.tile_pool(name="const", bufs=1) as cp, \
             tc.tile_pool(name="xgp", bufs=2) as xgp, \
             tc.tile_pool(name="wmp", bufs=2) as wmp, \
             tc.tile_pool(name="eqp", bufs=2) as eqp, \
             tc.tile_pool(name="t1p", bufs=2) as t1p, \
             tc.tile_pool(name="tlp", bufs=2) as tlp, \
             tc.tile_pool(name="stp", bufs=2) as stp, \
             tc.tile_pool(name="pst1", bufs=2, space="PSUM") as pst1, \
             tc.tile_pool(name="psy", bufs=2, space="PSUM") as psy:
            # small idx slice for slots 0-1 first so their gathers start
            # immediately; the big loads all queue behind them.
            S01 = int(meta[2][0]["io"])
            idx0_t = cp.tile([128, S01], i16)
            nc.sync.dma_start(out=idx0_t[:], in_=idxd.ap()[:, :S01])
            idx_t = cp.tile([128, ITOT], i16)
            nc.sync.dma_start(out=idx_t[:], in_=idxd.ap())
            zero3 = cp.tile([128, 3, 128], f8)
            nc.vector.memset(zero3[:], 0.0)
            mats_t = cp.tile([128, n_pairs * 128], f16)
            bias_t = cp.tile([128, 3], f32)
            iota_t = cp.tile([128, 128], f16)
            dm_t = cp.tile([128, JCOLTOT], f16)
            wh_t = cp.tile([128, JCOLTOT], f16)
            xt_t = cp.tile([128, 3, SLOTS * P], f16)

            # 7-slot output batches, with small final batches so the last
            # slots' compute tail overlaps earlier writes
            bstart = {0: 7, 7: 7, 14: 7, 21: 7, 28: 7, 35: 7,
                      42: 3, 45: 2, 47: 1, 48: 1}
            stage_t = None
            bs, bw = 0, 7
            for i in range(SLOTS):
                # one gather tile per call so matmuls only wait on their
                # own call's gather, not the whole slot's
                xgs = []
                for k, m in enumerate(meta[i]):
                    xg_t = xgp.tile([128, JCM[k], GELEM], f8, tag=f"xg{k}")
                    xgs.append(xg_t)
                    it = idx0_t if i < 2 else idx_t
                    nc.gpsimd.dma_gather(
                        xg_t[:, 0:m["jc"], :],
                        xrow8.ap()[m["h"] * B1:m["h"] * B1 + WIN, :],
                        it[:, m["io"]:m["io"] + (m["nexact"] + 15) // 16],
                        m["nexact"], m["nexact"], GELEM,
                        single_packet=False)
                if i == 0:
                    nc.sync.dma_start(out=dm_t[:], in_=dstld.ap())
                    nc.sync.dma_start(out=wh_t[:], in_=whatd.ap())
                    nc.sync.dma_start(out=iota_t[:], in_=iotad.ap())
                    nc.sync.dma_start(out=mats_t[:], in_=matsd.ap())
                    nc.sync.dma_start(out=bias_t[:], in_=biasd.ap())
                    nc.sync.dma_start(out=xt_t[:], in_=xTd.ap())
                if i in bstart:
                    bs, bw = i, bstart[i]
                    stage_t = stp.tile([128, 3, 7 * P], f16, tag="st")

                # one-hot * w_hat masks (fp16 so w_hat stays exact; the
                # message matmul mixes fp16 moving x fp8 stationary)
                wm_t = wmp.tile([128, WMX, 32], f16, tag="wm")
                woff = 0
                mm_by_col = {}  # (call, xg col, rows) -> list of (wm col, s)
                for ci, m in enumerate(meta[i]):
                    lastc = m["jc"] - 1
                    rem = m["nexact"] % 128
                    for s in range(m["slo"], m["shi"]):
                        nu = int(NU[i, m["h"], s])
                        if nu == 0:
                            continue
                        o0 = int(m["offs"][s - m["slo"]])
                        c0, c1 = o0 // 128, (o0 + nu - 1) // 128
                        wn = c1 - c0 + 1
                        a = int(m["co"]) + c0
                        eq_t = eqp.tile([128, WSMAX, 32], f16, tag="eq")
                        nc.vector.tensor_tensor(
                            out=eq_t[:, :wn, :],
                            in0=dm_t[:, a:a + wn].unsqueeze(2).to_broadcast([128, wn, 32]),
                            in1=iota_t[:, 32 * s:32 * s + 32].unsqueeze(1).to_broadcast([128, wn, 32]),
                            op=mybir.AluOpType.is_equal)
                        nc.vector.tensor_tensor(
                            out=wm_t[:, woff:woff + wn, :],
                            in0=eq_t[:, :wn, :],
                            in1=wh_t[:, a:a + wn].unsqueeze(2).to_broadcast([128, wn, 32]),
                            op=mybir.AluOpType.mult)
                        for q in range(wn):
                            xc = c0 + q
                            pr = rem if (xc == lastc and rem) else 128
                            mm_by_col.setdefault((ci, xc, pr), []).append((woff + q, s))
                        woff += wn

                # message aggregation, channel-major: t1T[ch, dst] in PSUM.
                # call-major / column-major order so matmuls chase gathers.
                pst = pst1.tile([128, 3, 128], f32, space="PSUM", tag="t1")
                nc.tensor.matmul(out=pst[:], lhsT=zero3[:, 0, :], rhs=zero3[:],
                                 start=True, stop=False, skip_group_check=True)
                cols = sorted(mm_by_col.items())
                total = 3 * sum(len(v) for _, v in cols)
                k = 0
                for (ci, xcol, pr), ws in cols:
                    for b in range(3):
                        for (wcol, s) in ws:
                            k += 1
                            nc.tensor.matmul(
                                out=pst[:, b, 32 * s:32 * s + 32],
                                lhsT=xgs[ci][0:pr, xcol, b * 128:(b + 1) * 128],
                                rhs=wm_t[0:pr, wcol, :],
                                start=False, stop=(k == total),
                                skip_group_check=True)

                t1sb = t1p.tile([128, 3, 128], f16, tag="t1sb")
                nc.scalar.mul(out=t1sb[:], in_=pst[:], mul=DS)

                # cheb + temporal-conv fold (channel-major y); separate psum
                # tile per go so act(go) doesn't serialize go+1's matmuls
                for go in range(3):
                    yps = psy.tile([128, 128], f32, space="PSUM", tag=f"y{go}")
                    plist = pairs_by_go[go]
                    for n_, (pi_, gi, path) in enumerate(plist):
                        rhs = (xt_t[:, gi, i * P:(i + 1) * P] if path == 0
                               else t1sb[:, gi, :])
                        nc.tensor.matmul(
                            out=yps[:],
                            lhsT=mats_t[:, pi_ * 128:(pi_ + 1) * 128],
                            rhs=rhs,
                            start=(n_ == 0), stop=(n_ == len(plist) - 1),
                            skip_group_check=True)
                    ysl = stage_t[:, go, (i - bs) * P:(i - bs + 1) * P]
                    nc.scalar.activation(out=ysl, in_=yps[:],
                                         func=mybir.ActivationFunctionType.Identity,
                                         bias=bias_t[:, go:go + 1], scale=1.0)
                    tl = tlp.tile([128, 128], f16, tag="tl")
                    nc.vector.tensor_scalar_mul(out=tl[:], in0=ysl, scalar1=0.01)
                    nc.vector.tensor_tensor(out=ysl, in0=ysl, in1=tl[:],
                                            op=mybir.AluOpType.max)

                if i == bs + bw - 1:
                    nc.sync.dma_start(
                        out=out_pc.ap()[:, :, bs * P:(bs + bw) * P],
                        in_=stage_t[:, :, 0:bw * P])

    nc.compile()
    return nc


def kernel(x, A, Ew, Wcheb, bcheb, Wconv, bconv, batch_size=1):
    from concourse.bass_utils import run_bass_kernel_spmd

    (xrow8, xT, idx16, dstl, what, newid,
     NU, meta, ITOT, JCOLTOT) = _host_prep(x, A, Ew)
    mats_sb, bias_sb, pairs = _fold_weights(Wcheb, bcheb, Wconv, bconv)

    key = (ITOT, JCOLTOT, tuple(NU.reshape(-1).tolist()))
    if key not in _cache:
        _cache[key] = _build_program(NU, meta, ITOT, JCOLTOT, len(pairs))
    nc = _cache[key]

    iota_np = np.tile(np.arange(128, dtype=np.float16)[None, :], (128, 1))
    in_maps = []
    for c in range(NCORES):
        in_maps.append(dict(
            xrow8=xrow8,
            xT=np.ascontiguousarray(xT[:, :, c * SLOTS * P:(c + 1) * SLOTS * P]),
            idx16=idx16[c], dstl=dstl[c], what=what[c],
            mats=mats_sb, biasd=bias_sb, iota=iota_np))
    res = run_bass_kernel_spmd(nc, in_maps, core_ids=list(range(NCORES)))
    # out_pc[c]: [128, 3, 6272] channel-major fp16 -> [50000, 12, 32] f32
    cols = [np.asarray(res.results[c]["out_pc"], np.float16).reshape(128, 3, SLOTS * P)
            for c in range(NCORES)]
    full = np.concatenate(cols, axis=2)                  # [128, 3, 50176]
    full = full.transpose(1, 0, 2).reshape(WC, NPAD)     # [384, 50176]
    y = np.ascontiguousarray(full[:, newid[:N]].T).astype(np.float32)
    return y.reshape(N, W, C)
